# revision 2
# baseline (speedup 1.0000x reference)
"""MoE-attention kernel for 8 Trainium2 NeuronCores.

Sharding: token-parallel. Core c handles sequence b = c//2, query-token half
half = c%2 (512 query tokens). Each core computes all 20 experts for its
query tokens; K/V context is the full 1024-token sequence, fed with the
local half FIRST (attention is permutation-invariant over key positions).
No collectives: out_proj partial sums are avoided by giving every core the
full feature dim for its own query tokens.

Layout strategy (all contractions need the contracted dim on partitions):
  xT       [128d, 10dt, 1024t]   PE-transposed input
  qkT      [128(q|k), 1024t]     = wqk[d,128].T @ xT  (+bias via rank-1 matmul)
  scoresT  [128kt, 512qt]        = kT[h,kt].T @ qT[h,qt]
  attnT    = exp(scoresT*scale)  (no max-subtraction: |scores*scale| < ~3)
  v        [128kt, 65]           natural layout + ones column -> sumexp row
  eoT      [65h, 512qt]          = v.T @ attnT ; row 64 = sumexp
  combT    [128h', 10, 512qt]    = eoT * gate/sumexp (expert e -> h'=e*64..)
  out      [512t, 1280]          = combT.T @ out_w + out_b
"""

import numpy as np

import concourse.bass as bass
import concourse.mybir as mybir
import concourse.tile as tile
from concourse import bacc
from concourse.masks import make_identity
from concourse.bass_utils import run_bass_kernel_spmd

import os
F32 = mybir.dt.float32
F32R = mybir.dt.float32r
BF16 = mybir.dt.bfloat16
MM_DT = BF16 if os.environ.get("KERNEL_MM_DT") == "bf16" else F32R
try:
    import ml_dtypes as _mld
    _BF16_NP = np.dtype(_mld.bfloat16)
except Exception:
    _BF16_NP = np.float32
NP_MM = _BF16_NP if os.environ.get("KERNEL_MM_DT") == "bf16" else np.float32
AF = mybir.ActivationFunctionType

B = 4
S = 1024          # sequence length (full context per core)
D = 1280          # d_model
E = 20            # experts
H = 64            # head dim
SL = 512          # local query tokens per core
DT = D // 128     # 10 d-tiles
HT = D // 128     # 10 h'-tiles
KT = S // 128     # 8 key tiles
SCALE = float(H) ** -0.5
NCORES = 8
GROUPS = [(0, 8), (8, 16), (16, 20)]
OCH = 256         # out_proj column chunk (>=256 keeps f32r at full rate)


def _r(ap):
    return ap


def _mm(nc, out, lhsT, rhs, **kw):
    nc.tensor.matmul(out, _r(lhsT), _r(rhs), **kw)


def _emit(tc, x_d, wq_d, wk_d, bq_d, bk_d, wv_d, bv_d,
          rw_d, rb_d, ow_d, ob_d, out_d):
    nc = tc.nc
    with (
        tc.tile_pool(name="const", bufs=1) as const,
        tc.tile_pool(name="io", bufs=2) as io,
        tc.tile_pool(name="wq", bufs=2) as wqp,
        tc.tile_pool(name="wv", bufs=1) as wvp,
        tc.tile_pool(name="vp", bufs=1) as vp,
        tc.tile_pool(name="qk", bufs=2) as qkp,
        tc.tile_pool(name="at", bufs=4) as atp,
        tc.tile_pool(name="sm", bufs=2) as smp,
        tc.tile_pool(name="ow", bufs=2) as owp,
        tc.tile_pool(name="pp", bufs=1, space="PSUM") as pp,
        tc.tile_pool(name="ps", bufs=4, space="PSUM") as psp,
        tc.tile_pool(name="pe", bufs=2, space="PSUM") as pep,
    ):
        # ---- constants ----
        ident = const.tile([128, 128], F32, name="ident")
        make_identity(nc, ident)
        ones_f32 = const.tile([128, 512], F32, name="ones_f32")
        nc.gpsimd.memset(ones_f32, 1.0)
        ones_row = const.tile([1, 512], MM_DT, name="ones_row")
        nc.vector.tensor_copy(ones_row, ones_f32[0:1, :])
        ones20 = const.tile([E, 1], MM_DT, name="ones20")
        nc.vector.tensor_copy(ones20, ones_f32[0:E, 0:1])

        xT = const.tile([128, DT, S], MM_DT, name="xT")
        combT = const.tile([128, HT, SL], MM_DT, name="combT")

        rw_sb = const.tile([128, DT, E], MM_DT, name="rw_sb")
        nc.sync.dma_start(out=rw_sb, in_=rw_d.rearrange("(t p) e -> p t e", p=128))
        rb_sb = const.tile([1, E], MM_DT, name="rb_sb")
        nc.sync.dma_start(out=rb_sb, in_=rb_d[None, :])
        ob_sb = const.tile([1, D], MM_DT, name="ob_sb")
        nc.sync.dma_start(out=ob_sb, in_=ob_d[None, :])

        exp_router = const.tile([E, SL], MM_DT, name="exp_router")
        gates_sb = const.tile([E, SL], F32, name="gates_sb")
        inv_rsum = const.tile([1, SL], F32, name="inv_rsum")
        inv_rep = const.tile([E, SL], F32, name="inv_rep")

        # ---- phase 1: transpose x into xT ----
        for tt in range(KT):
            x_ch = io.tile([128, D], F32, name="x_ch", tag="x_ch")
            xrow = x_d[tt * 128:(tt + 1) * 128, :]
            nc.sync.dma_start(out=x_ch[:, 0:D // 2], in_=xrow[:, 0:D // 2])
            nc.scalar.dma_start(out=x_ch[:, D // 2:], in_=xrow[:, D // 2:])
            for dt in range(DT):
                tp = psp.tile([128, 128], F32, name="tp", tag="sc")
                nc.tensor.matmul(
                    tp, x_ch[:, dt * 128:(dt + 1) * 128], ident,
                    is_transpose=True, start=True, stop=True,
                )
                nc.vector.tensor_copy(xT[:, dt, tt * 128:(tt + 1) * 128], tp)

        # ---- phase 2: router softmax gates for local query tokens ----
        rt_ps = psp.tile([E, SL], F32, name="rt_ps", tag="sc")
        for dt in range(DT):
            _mm(nc, rt_ps, rw_sb[:, dt, :], xT[:, dt, 0:SL],
                start=(dt == 0), stop=False)
        _mm(nc, rt_ps, rb_sb, ones_row, start=False, stop=True)
        nc.scalar.activation(exp_router, rt_ps, AF.Exp)
        rs_ps = psp.tile([1, SL], F32, name="rs_ps", tag="sc")
        _mm(nc, rs_ps, ones20, exp_router, start=True, stop=True)
        nc.vector.reciprocal(inv_rsum, rs_ps)
        nc.gpsimd.partition_broadcast(inv_rep, inv_rsum)
        nc.vector.tensor_mul(gates_sb,
                             exp_router.bitcast(F32) if MM_DT is F32R else exp_router,
                             inv_rep)

        # ---- phase 3: expert groups (paired q/k) ----
        for (g0, g1) in GROUPS:
            gsz = g1 - g0
            wv_sb = wvp.tile([128, DT, gsz * H], MM_DT, name="wv_sb", tag="wv")
            hw = gsz * H // 2
            wv_in = wv_d[:, g0 * H:g1 * H].rearrange("(t p) h -> p t h", p=128)
            nc.sync.dma_start(out=wv_sb[:, :, 0:hw], in_=wv_in[:, :, 0:hw])
            nc.scalar.dma_start(out=wv_sb[:, :, hw:], in_=wv_in[:, :, hw:])
            bv_row = wvp.tile([1, gsz * H], MM_DT, name="bv_row", tag="bv")
            nc.sync.dma_start(
                out=bv_row, in_=bv_d[:, g0 * H:g1 * H])
            v_sb = vp.tile([128, KT, gsz, H + 1], MM_DT, name="v_sb", tag="vg")
            nc.vector.tensor_copy(
                v_sb[:, :, :, H],
                ones_f32[:, 0:KT * gsz].rearrange("p (a b) -> p a b", a=KT),
            )
            for tt in range(KT):
                v_ps = psp.tile([128, gsz * H], F32, name="v_ps", tag="sc")
                for dt in range(DT):
                    _mm(nc, v_ps, xT[:, dt, tt * 128:(tt + 1) * 128], wv_sb[:, dt, :],
                        start=(dt == 0), stop=False)
                _mm(nc, v_ps, ones_row[:, 0:128], bv_row, start=False, stop=True)
                nc.vector.tensor_copy(
                    v_sb[:, tt, :, 0:H],
                    v_ps.rearrange("p (e h) -> p e h", e=gsz),
                )

            for p in range(g0 // 2, g1 // 2):
                wq_sb = wqp.tile([128, DT, 128], MM_DT, name="wq_sb", tag="wq")
                nc.sync.dma_start(
                    out=wq_sb,
                    in_=wq_d[p].rearrange("(t p) h -> p t h", p=128))
                wk_sb = wqp.tile([128, DT, 128], MM_DT, name="wk_sb", tag="wk")
                nc.scalar.dma_start(
                    out=wk_sb,
                    in_=wk_d[p].rearrange("(t p) h -> p t h", p=128))
                bq_row = wqp.tile([1, 128], MM_DT, name="bq_row", tag="bq")
                nc.sync.dma_start(out=bq_row, in_=bq_d[p, None, :])
                bk_row = wqp.tile([1, 128], MM_DT, name="bk_row", tag="bk")
                nc.sync.dma_start(out=bk_row, in_=bk_d[p, None, :])

                kt_ps = pp.tile([128, S], F32, name="kt_ps", tag="big")
                for ch in range(2):
                    sl = slice(ch * 512, (ch + 1) * 512)
                    for dt in range(DT):
                        _mm(nc, kt_ps[:, sl], wk_sb[:, dt, :], xT[:, dt, sl],
                            start=(dt == 0), stop=False)
                    _mm(nc, kt_ps[:, sl], bk_row, ones_row, start=False, stop=True)
                k_sb = qkp.tile([128, S], MM_DT, name="k_sb", tag="k")
                nc.vector.tensor_copy(k_sb, kt_ps)

                qt_ps = psp.tile([128, SL], F32, name="qt_ps", tag="sc")
                for dt in range(DT):
                    _mm(nc, qt_ps, wq_sb[:, dt, :], xT[:, dt, 0:SL],
                        start=(dt == 0), stop=False)
                _mm(nc, qt_ps, bq_row, ones_row, start=False, stop=True)
                q_sb = qkp.tile([128, SL], MM_DT, name="q_sb", tag="q")
                nc.vector.tensor_copy(q_sb, qt_ps)

                for sub in range(2):
                    e = 2 * p + sub
                    i = e - g0
                    po = sub * 64
                    eo_ps = pep.tile([H + 1, SL], F32, name="eo_ps", tag="eo")
                    for kt in range(KT):
                        sc_ps = psp.tile([128, SL], F32, name="sc_ps", tag="sc")
                        _mm(nc, sc_ps, k_sb[po:po + 64, kt * 128:(kt + 1) * 128],
                            q_sb[po:po + 64, :], start=True, stop=True)
                        at_sb = atp.tile([128, SL], MM_DT, name="at_sb", tag="at")
                        nc.scalar.activation(at_sb, sc_ps, AF.Exp, scale=SCALE)
                        _mm(nc, eo_ps, v_sb[:, kt, i, :], at_sb,
                            start=(kt == 0), stop=(kt == KT - 1))

                    # gate/sumexp normalization of eoT -> combT
                    s_inv = smp.tile([1, SL], F32, name="s_inv")
                    nc.vector.reciprocal(s_inv, eo_ps[H:H + 1, :])
                    g_row0 = smp.tile([1, SL], F32, name="g_row0")
                    nc.sync.dma_start(out=g_row0, in_=gates_sb[e:e + 1, :])
                    g_row = smp.tile([1, SL], F32, name="g_row")
                    nc.vector.tensor_mul(g_row, s_inv, g_row0)
                    sc64 = smp.tile([H, SL], F32, name="sc64")
                    nc.gpsimd.partition_broadcast(sc64, g_row)
                    cpo = (e % 2) * 64
                    nc.vector.tensor_mul(
                        combT[cpo:cpo + 64, e // 2, :], eo_ps[0:H, :], sc64
                    )

        # ---- phase 4: out projection ----
        for ch in range(D // OCH):
            ow_sb = owp.tile([128, HT, OCH], MM_DT, name="ow_sb", tag="ow")
            ow_in = ow_d[:, ch * OCH:(ch + 1) * OCH].rearrange(
                "(t p) n -> p t n", p=128)
            nc.sync.dma_start(out=ow_sb[:, 0:HT // 2, :],
                              in_=ow_in[:, 0:HT // 2, :])
            nc.scalar.dma_start(out=ow_sb[:, HT // 2:, :],
                                in_=ow_in[:, HT // 2:, :])
            for tt in range(SL // 128):
                op_ps = pep.tile([128, OCH], F32, name="op_ps", tag="eo")
                for ht in range(HT):
                    _mm(nc, op_ps, combT[:, ht, tt * 128:(tt + 1) * 128],
                        ow_sb[:, ht, :], start=(ht == 0), stop=False)
                _mm(nc, op_ps, ones_row[:, 0:128],
                    ob_sb[:, ch * OCH:(ch + 1) * OCH], start=False, stop=True)
                o_sb = io.tile([128, OCH], F32, name="o_sb", tag="o_sb")
                nc.vector.tensor_copy(o_sb, op_ps)
                nc.sync.dma_start(
                    out=out_d[tt * 128:(tt + 1) * 128, ch * OCH:(ch + 1) * OCH],
                    in_=o_sb,
                )


def declare_tensors(nc):
    x_d = nc.dram_tensor("x_ctx", [S, D], F32, kind="ExternalInput").ap()
    wq_d = nc.dram_tensor("wq", [E // 2, D, 128], MM_DT, kind="ExternalInput").ap()
    wk_d = nc.dram_tensor("wk", [E // 2, D, 128], MM_DT, kind="ExternalInput").ap()
    bq_d = nc.dram_tensor("bq", [E // 2, 128], MM_DT, kind="ExternalInput").ap()
    bk_d = nc.dram_tensor("bk", [E // 2, 128], MM_DT, kind="ExternalInput").ap()
    wv_d = nc.dram_tensor("wv", [D, E * H], MM_DT, kind="ExternalInput").ap()
    bv_d = nc.dram_tensor("bv", [1, E * H], MM_DT, kind="ExternalInput").ap()
    rw_d = nc.dram_tensor("router_w", [D, E], MM_DT, kind="ExternalInput").ap()
    rb_d = nc.dram_tensor("router_b", [E], MM_DT, kind="ExternalInput").ap()
    ow_d = nc.dram_tensor("out_w", [D, D], MM_DT, kind="ExternalInput").ap()
    ob_d = nc.dram_tensor("out_b", [D], MM_DT, kind="ExternalInput").ap()
    out_d = nc.dram_tensor("out", [SL, D], F32, kind="ExternalOutput").ap()
    return (x_d, wq_d, wk_d, bq_d, bk_d, wv_d, bv_d,
            rw_d, rb_d, ow_d, ob_d, out_d)


def build_nc():
    nc = bacc.Bacc("TRN2", target_bir_lowering=False, debug=False,
                   num_devices=NCORES)
    tensors = declare_tensors(nc)
    with tile.TileContext(nc) as tc:
        _emit(tc, *tensors)
    nc.compile()
    return nc


_NC = None


def _get_nc():
    global _NC
    if _NC is None:
        _NC = build_nc()
    return _NC


def make_in_maps(x, wqkv, bqkv, router_w, router_b, out_w, out_b):
    x = np.ascontiguousarray(np.asarray(x, np.float32))
    wqkv = np.asarray(wqkv, np.float32)
    bqkv = np.asarray(bqkv, np.float32)
    wq = np.ascontiguousarray(
        wqkv[:, :, 0:H].reshape(E // 2, 2, D, H).transpose(0, 2, 1, 3)
        .reshape(E // 2, D, 128))
    wk = np.ascontiguousarray(
        wqkv[:, :, H:2 * H].reshape(E // 2, 2, D, H).transpose(0, 2, 1, 3)
        .reshape(E // 2, D, 128))
    bq = np.ascontiguousarray(bqkv[:, 0:H].reshape(E // 2, 128))
    bk = np.ascontiguousarray(bqkv[:, H:2 * H].reshape(E // 2, 128))
    wv = np.ascontiguousarray(
        wqkv[:, :, 2 * H:3 * H].transpose(1, 0, 2).reshape(D, E * H))
    bv = np.ascontiguousarray(bqkv[:, 2 * H:3 * H].reshape(1, E * H))
    def _c(a):
        return np.ascontiguousarray(np.asarray(a, np.float32).astype(NP_MM))
    shared = {
        "wq": _c(wq), "wk": _c(wk), "bq": _c(bq), "bk": _c(bk),
        "wv": _c(wv), "bv": _c(bv),
        "router_w": _c(router_w), "router_b": _c(router_b),
        "out_w": _c(out_w), "out_b": _c(out_b),
    }
    in_maps = []
    for c in range(NCORES):
        b, half = c // 2, c % 2
        xb = x[b]
        if half == 0:
            x_ctx = xb
        else:
            x_ctx = np.ascontiguousarray(
                np.concatenate([xb[SL:], xb[:SL]], axis=0))
        in_maps.append({"x_ctx": x_ctx, **shared})
    return in_maps


def gather_out(results):
    out = np.empty((B, S, D), np.float32)
    for c in range(NCORES):
        b, half = c // 2, c % 2
        out[b, half * SL:(half + 1) * SL] = results[c]["out"]
    return out


def kernel(x, wqkv, bqkv, router_w, router_b, out_w, out_b):
    nc = _get_nc()
    in_maps = make_in_maps(x, wqkv, bqkv, router_w, router_b, out_w, out_b)
    res = run_bass_kernel_spmd(nc, in_maps, core_ids=list(range(NCORES)))
    return gather_out(res.results)



# revision 16
# speedup vs baseline: 1.0954x; 1.0954x over previous
"""MoE-attention kernel for 8 Trainium2 NeuronCores.

Sharding: token-parallel. Core c handles sequence b = c//2, query-token half
half = c%2 (512 query tokens). Each core computes all 20 experts for its
query tokens; K/V context is the full 1024-token sequence, fed with the
local half FIRST (attention is permutation-invariant over key positions).
No collectives: out_proj partial sums are avoided by giving every core the
full feature dim for its own query tokens.

v2 layout strategy (bf16 matmuls, f32 PSUM accumulate):
  xT       [128d, 10dt, 1024t]   loaded pre-transposed from host (no PE pass)
  all weights DMA'd up-front in a few large transfers (no per-pair DMAs)
  qkT      [128(q|k), St]        = wqk[d,128].T @ xT  (+bias rank-1 matmul)
  scoresT  [128kt, 2, 512qt]     2 key-tiles per PSUM tile -> one big exp
  attnT    = exp(scoresT*scale)  (no max-subtraction: |scores*scale| < ~3)
  pipeline: exp(unit u) on Act overlaps eo-matmuls(u-1) on PE
  v        [128t, kt, e, 65]     natural layout + ones column -> sumexp row
  eoT      [65h, 512qt]          = v.T @ attnT ; row 64 = sumexp
  combT    [128h', 10, 512qt]    = eoT * gate/sumexp (expert e -> h'=e*64..)
  out      [512t, 1280]          = combT.T @ out_w + out_b  (512-col chunks)
"""

import numpy as np

import concourse.bass as bass
import concourse.mybir as mybir
import concourse.tile as tile
from concourse import bacc
from concourse.bass_utils import run_bass_kernel_spmd

F32 = mybir.dt.float32
BF16 = mybir.dt.bfloat16
MM_DT = BF16
try:
    import ml_dtypes as _mld
    NP_MM = np.dtype(_mld.bfloat16)
except Exception as e:  # pragma: no cover
    raise RuntimeError("ml_dtypes required for bf16 host packing") from e
AF = mybir.ActivationFunctionType

B = 4
S = 1024          # sequence length (full context per core)
D = 1280          # d_model
E = 20            # experts
EP = E // 2       # expert pairs
H = 64            # head dim
SL = 512          # local query tokens per core
DT = D // 128     # 10 d-tiles
HT = D // 128     # 10 h'-tiles
KT = S // 128     # 8 key tiles
SCALE = float(H) ** -0.5
NCORES = 8
GROUPS = [(0, 8), (8, 16), (16, 20)]
OCHUNKS = [(0, 512), (512, 1024), (1024, 1280)]


def _mm(nc, out, lhsT, rhs, **kw):
    nc.tensor.matmul(out, lhsT, rhs, **kw)


def _emit(tc, xT_d, wqk_d, bqk_d, wv_d, bv_d, rw_d, rb_d, ow_d, ob_d, out_d):
    nc = tc.nc
    with (
        tc.tile_pool(name="const", bufs=1) as const,
        tc.tile_pool(name="io", bufs=2) as io,
        tc.tile_pool(name="vp", bufs=2) as vp,
        tc.tile_pool(name="qk", bufs=2) as qkp,
        tc.tile_pool(name="at", bufs=4) as atp,
        tc.tile_pool(name="sm", bufs=2) as smp,
        tc.tile_pool(name="ps", bufs=3, space="PSUM") as psp,
        tc.tile_pool(name="pe", bufs=2, space="PSUM") as pep,
    ):
        # ---- constants + all weights up-front (few large DMAs) ----
        ones_f32 = const.tile([128, 512], F32, name="ones_f32")
        nc.gpsimd.memset(ones_f32, 1.0)
        ones_row = const.tile([1, 512], MM_DT, name="ones_row")
        nc.vector.tensor_copy(ones_row, ones_f32[0:1, :])
        ones20 = const.tile([E, 1], MM_DT, name="ones20")
        nc.vector.tensor_copy(ones20, ones_f32[0:E, 0:1])

        xT = const.tile([128, DT, S], MM_DT, name="xT")
        xin = xT_d.rearrange("(t p) s -> p t s", p=128)
        nc.sync.dma_start(out=xT[:, 0:DT // 2, :], in_=xin[:, 0:DT // 2, :])
        nc.scalar.dma_start(out=xT[:, DT // 2:, :], in_=xin[:, DT // 2:, :])

        rw_sb = const.tile([128, DT, E], MM_DT, name="rw_sb")
        nc.sync.dma_start(out=rw_sb, in_=rw_d.rearrange("(t p) e -> p t e", p=128))
        rb_sb = const.tile([1, E], MM_DT, name="rb_sb")
        nc.sync.dma_start(out=rb_sb, in_=rb_d[None, :])

        wv_sb = const.tile([128, DT, E * H], MM_DT, name="wv_sb")
        wv_in = wv_d.rearrange("(t p) h -> p t h", p=128)
        nc.sync.dma_start(out=wv_sb[:, :, 0:8 * H], in_=wv_in[:, :, 0:8 * H])
        nc.scalar.dma_start(out=wv_sb[:, :, 8 * H:], in_=wv_in[:, :, 8 * H:])
        bv_row = const.tile([1, E * H], MM_DT, name="bv_row")
        nc.sync.dma_start(out=bv_row, in_=bv_d)

        wqk_sb = const.tile([128, EP, DT, 256], MM_DT, name="wqk_sb")
        wqk_in = wqk_d.rearrange("e (t p) h -> p e t h", p=128)
        nc.sync.dma_start(out=wqk_sb[:, 0:EP // 2], in_=wqk_in[:, 0:EP // 2])
        nc.scalar.dma_start(out=wqk_sb[:, EP // 2:], in_=wqk_in[:, EP // 2:])
        bqk_sb = const.tile([1, EP, 256], MM_DT, name="bqk_sb")
        nc.sync.dma_start(out=bqk_sb, in_=bqk_d[None, :, :])

        ow_sb = const.tile([128, DT, D], MM_DT, name="ow_sb")
        ow_in = ow_d.rearrange("(t p) n -> p t n", p=128)
        nc.sync.dma_start(out=ow_sb[:, 0:DT // 2], in_=ow_in[:, 0:DT // 2])
        nc.scalar.dma_start(out=ow_sb[:, DT // 2:], in_=ow_in[:, DT // 2:])
        ob_sb = const.tile([1, D], MM_DT, name="ob_sb")
        nc.sync.dma_start(out=ob_sb, in_=ob_d[None, :])

        combT = const.tile([128, HT, SL], MM_DT, name="combT")

        # ---- router logits for local query tokens (gates finished after
        # group-0 V so the PE never waits on the Act queue's DMA issues) ----
        exp_router = const.tile([E, SL], MM_DT, name="exp_router")
        gates_sb = const.tile([E, SL], F32, name="gates_sb")
        inv_rsum = const.tile([1, SL], F32, name="inv_rsum")
        inv_rep = const.tile([E, SL], F32, name="inv_rep")

        rt_ps = psp.tile([E, SL], F32, name="rt_ps", tag="sc")
        for dt in range(DT):
            _mm(nc, rt_ps, rw_sb[:, dt, :], xT[:, dt, 0:SL],
                start=(dt == 0), stop=False)
        _mm(nc, rt_ps, rb_sb, ones_row, start=False, stop=True)
        nc.scalar.activation(exp_router, rt_ps, AF.Exp)

        def finish_gates():
            rs_ps = psp.tile([1, SL], F32, name="rs_ps", tag="sc")
            _mm(nc, rs_ps, ones20, exp_router, start=True, stop=True)
            nc.vector.reciprocal(inv_rsum, rs_ps)
            nc.gpsimd.partition_broadcast(inv_rep, inv_rsum)
            nc.vector.tensor_mul(gates_sb, exp_router, inv_rep)

        # ---- expert groups ----
        # normalize of pair p is deferred behind pair p+1's q/k copies so the
        # in-order DVE queue never delays the next pair's first scores matmul
        pending_norm = [None]

        def make_normalize(eo_tiles, p, g0_tiles):
            def norm():
                for sub in range(2):
                    e = 2 * p + sub
                    eo_ps = eo_tiles[sub]
                    s_inv = smp.tile([1, SL], F32, name="s_inv")
                    nc.vector.reciprocal(s_inv, eo_ps[H:H + 1, :])
                    g_row = smp.tile([1, SL], F32, name="g_row")
                    nc.vector.tensor_mul(g_row, s_inv, g0_tiles[sub])
                    sc64 = smp.tile([H, SL], F32, name="sc64")
                    nc.gpsimd.partition_broadcast(sc64, g_row)
                    cpo = (e % 2) * 64
                    nc.vector.tensor_mul(
                        combT[cpo:cpo + 64, e // 2, :], eo_ps[0:H, :], sc64
                    )
            return norm

        first_group = True
        for (g0, g1) in GROUPS:
            gsz = g1 - g0
            v_sb = vp.tile([128, KT, gsz, H + 1], MM_DT, name="v_sb", tag="vg")
            nc.vector.tensor_copy(
                v_sb[:, :, :, H],
                ones_f32[:, 0:KT * gsz].rearrange("p (a b) -> p a b", a=KT),
            )
            for tt in range(KT):
                v_ps = psp.tile([128, gsz * H], F32, name="v_ps", tag="sc")
                for dt in range(DT):
                    _mm(nc, v_ps, xT[:, dt, tt * 128:(tt + 1) * 128],
                        wv_sb[:, dt, g0 * H:g1 * H],
                        start=(dt == 0), stop=False)
                _mm(nc, v_ps, ones_row[:, 0:128], bv_row[:, g0 * H:g1 * H],
                    start=False, stop=True)
                nc.vector.tensor_copy(
                    v_sb[:, tt, :, 0:H],
                    v_ps.rearrange("p (e h) -> p e h", e=gsz),
                )
            if first_group:
                finish_gates()
                first_group = False

            for p in range(g0 // 2, g1 // 2):
                # Q^T first so its copy hides under the K^T chains; K^T
                # copied per 512-chunk so chunk 1's chain hides chunk 0's copy
                qt_ps = psp.tile([128, SL], F32, name="qt_ps", tag="sc")
                for dt in range(DT):
                    _mm(nc, qt_ps, wqk_sb[:, p, dt, 0:128], xT[:, dt, 0:SL],
                        start=(dt == 0), stop=False)
                _mm(nc, qt_ps, bqk_sb[:, p, 0:128], ones_row,
                    start=False, stop=True)
                q_sb = qkp.tile([128, SL], MM_DT, name="q_sb", tag="q")
                nc.vector.tensor_copy(q_sb, qt_ps)

                k_chunks = []
                for ch in range(2):
                    sl = slice(ch * 512, (ch + 1) * 512)
                    kt_ps = psp.tile([128, 512], F32, name=f"kt_ps{ch}",
                                     tag="sc")
                    for dt in range(DT):
                        _mm(nc, kt_ps, wqk_sb[:, p, dt, 128:256],
                            xT[:, dt, sl], start=(dt == 0), stop=False)
                    _mm(nc, kt_ps, bqk_sb[:, p, 128:256], ones_row,
                        start=False, stop=True)
                    k_sb = qkp.tile([128, 512], MM_DT, name=f"k_sb{ch}",
                                    tag=f"k{ch}")
                    nc.vector.tensor_copy(k_sb, kt_ps)
                    k_chunks.append(k_sb)

                if pending_norm[0] is not None:
                    pending_norm[0]()
                g0_tiles = []
                for sub in range(2):
                    g_row0 = smp.tile([1, SL], F32, name="g_row0")
                    nc.sync.dma_start(out=g_row0,
                                      in_=gates_sb[2 * p + sub:2 * p + sub + 1, :])
                    g0_tiles.append(g_row0)

                # attention: units (chunk c of 2 key-tiles, sub-expert s);
                # exp of unit u overlaps eo-matmuls of unit u-1 on PE
                eo_tiles = [
                    pep.tile([H + 1, SL], F32, name=f"eo_ps{s}", tag="eo")
                    for s in range(2)
                ]

                def emit_eo(at2, sub, c, i):
                    for j in range(2):
                        _mm(nc, eo_tiles[sub], v_sb[:, 2 * c + j, i, :],
                            at2[:, j, :],
                            start=(c == 0 and j == 0),
                            stop=(c == KT // 2 - 1 and j == 1))

                pending = []
                for c in range(KT // 2):
                    for sub in range(2):
                        po = sub * 64
                        i = 2 * p + sub - g0
                        sc2 = psp.tile([128, 2, SL], F32, name="sc2", tag="sc")
                        for j in range(2):
                            kt = 2 * c + j
                            _mm(nc, sc2[:, j, :],
                                k_chunks[kt // 4][po:po + 64,
                                                  (kt % 4) * 128:(kt % 4 + 1) * 128],
                                q_sb[po:po + 64, :], start=True, stop=True)
                        at2 = atp.tile([128, 2, SL], MM_DT, name="at2", tag="at")
                        nc.scalar.activation(at2, sc2, AF.Exp, scale=SCALE)
                        pending.append((at2, sub, c, i))
                        if len(pending) > 1:
                            emit_eo(*pending.pop(0))
                emit_eo(*pending.pop(0))

                pending_norm[0] = make_normalize(eo_tiles, p, g0_tiles)

        pending_norm[0]()

        # ---- out projection ----
        for tt in range(SL // 128):
            o_sb = io.tile([128, D], F32, name="o_sb", tag="o_sb")
            for (c0, c1) in OCHUNKS:
                op_ps = pep.tile([128, c1 - c0], F32, name="op_ps", tag="eo")
                for ht in range(HT):
                    _mm(nc, op_ps, combT[:, ht, tt * 128:(tt + 1) * 128],
                        ow_sb[:, ht, c0:c1], start=(ht == 0), stop=False)
                _mm(nc, op_ps, ones_row[:, 0:128], ob_sb[:, c0:c1],
                    start=False, stop=True)
                nc.vector.tensor_copy(o_sb[:, c0:c1], op_ps)
            nc.sync.dma_start(
                out=out_d[tt * 128:(tt + 1) * 128, :], in_=o_sb)


def declare_tensors(nc):
    xT_d = nc.dram_tensor("xT", [D, S], MM_DT, kind="ExternalInput").ap()
    wqk_d = nc.dram_tensor("wqk", [EP, D, 256], MM_DT, kind="ExternalInput").ap()
    bqk_d = nc.dram_tensor("bqk", [EP, 256], MM_DT, kind="ExternalInput").ap()
    wv_d = nc.dram_tensor("wv", [D, E * H], MM_DT, kind="ExternalInput").ap()
    bv_d = nc.dram_tensor("bv", [1, E * H], MM_DT, kind="ExternalInput").ap()
    rw_d = nc.dram_tensor("router_w", [D, E], MM_DT, kind="ExternalInput").ap()
    rb_d = nc.dram_tensor("router_b", [E], MM_DT, kind="ExternalInput").ap()
    ow_d = nc.dram_tensor("out_w", [D, D], MM_DT, kind="ExternalInput").ap()
    ob_d = nc.dram_tensor("out_b", [D], MM_DT, kind="ExternalInput").ap()
    out_d = nc.dram_tensor("out", [SL, D], F32, kind="ExternalOutput").ap()
    return (xT_d, wqk_d, bqk_d, wv_d, bv_d, rw_d, rb_d, ow_d, ob_d, out_d)


def build_nc():
    nc = bacc.Bacc("TRN2", target_bir_lowering=False, debug=False,
                   num_devices=NCORES)
    tensors = declare_tensors(nc)
    with tile.TileContext(nc) as tc:
        _emit(tc, *tensors)
    nc.compile()
    return nc


_NC = None


def _get_nc():
    global _NC
    if _NC is None:
        _NC = build_nc()
    return _NC


def make_in_maps(x, wqkv, bqkv, router_w, router_b, out_w, out_b):
    x = np.ascontiguousarray(np.asarray(x, np.float32))
    wqkv = np.asarray(wqkv, np.float32)
    bqkv = np.asarray(bqkv, np.float32)
    wq = wqkv[:, :, 0:H].reshape(EP, 2, D, H).transpose(0, 2, 1, 3).reshape(EP, D, 128)
    wk = wqkv[:, :, H:2 * H].reshape(EP, 2, D, H).transpose(0, 2, 1, 3).reshape(EP, D, 128)
    wqk = np.concatenate([wq, wk], axis=-1)
    bq = bqkv[:, 0:H].reshape(EP, 128)
    bk = bqkv[:, H:2 * H].reshape(EP, 128)
    bqk = np.concatenate([bq, bk], axis=-1)
    wv = wqkv[:, :, 2 * H:3 * H].transpose(1, 0, 2).reshape(D, E * H)
    bv = bqkv[:, 2 * H:3 * H].reshape(1, E * H)

    def _c(a):
        return np.ascontiguousarray(np.asarray(a, np.float32).astype(NP_MM))
    shared = {
        "wqk": _c(wqk), "bqk": _c(bqk), "wv": _c(wv), "bv": _c(bv),
        "router_w": _c(router_w), "router_b": _c(router_b),
        "out_w": _c(out_w), "out_b": _c(out_b),
    }
    in_maps = []
    for c in range(NCORES):
        b, half = c // 2, c % 2
        xb = x[b]
        if half == 0:
            x_ctx = xb
        else:
            x_ctx = np.concatenate([xb[SL:], xb[:SL]], axis=0)
        in_maps.append({"xT": _c(x_ctx.T), **shared})
    return in_maps


def gather_out(results):
    out = np.empty((B, S, D), np.float32)
    for c in range(NCORES):
        b, half = c // 2, c % 2
        out[b, half * SL:(half + 1) * SL] = results[c]["out"]
    return out


def kernel(x, wqkv, bqkv, router_w, router_b, out_w, out_b):
    nc = _get_nc()
    in_maps = make_in_maps(x, wqkv, bqkv, router_w, router_b, out_w, out_b)
    res = run_bass_kernel_spmd(nc, in_maps, core_ids=list(range(NCORES)))
    return gather_out(res.results)
